# revision 48
# baseline (speedup 1.0000x reference)
"""BaseAttentionPooling Trainium2 kernel.

reference:
    h = tanh(x @ W1 + b1)            # [N, H]
    logits = (h @ W2 + b2)[:, 0]     # [N]
    per-graph softmax over sorted `batch`, pooled = seg_sum(x * w)  # [G, D]

Strategy (data-parallel over graphs, 8 cores, SPMD-identical program):
  - 512 graphs/core, split into 4 blocks of 128 graphs.
  - Host pads each (core, block)'s nodes to `cpb` chunks of 128 nodes
    (cpb = max over all core/blocks, so the program is core-uniform).
  - b2 is dropped: it cancels in the softmax. exp() without max-subtraction:
    |logits| <= ||W2||_1·1 + |b2| is small, exp() safe in fp32.
  - Host ships BOTH x layouts: node-major bf16 (pooling moving operand,
    with a 257th ones column so the denominator rides the pooled matmul)
    and d-major fp8 (MLP moving operand, 2 k-tile planes for DoubleRow).
  - Per g8 group (8 chunks = 1024 nodes):
      hT = W1.T @ xT via 2 fp8 DoubleRow matmuls (K=256 folded, N=512 each)
      tanh(+b1, x1/W1SCALE) on ACT -> hsb bf16
      logits: 8 matmuls (hsb chunk stationary, W2 moving, N=1)
      exp -> e8 [128, 8]
      oh[i,g] = (iota==rel_gid[i])*e[i] built alternately on DVE / GpSimd
      pooled_blk[g, 0:257] += oh.T @ [x | 1] accumulated in PSUM over the
      block's chunks (col 256 = denominator), flushed one g8 late so the
      PE pipeline stays fed.
  - Epilogue: out[g] = pooled[g, 0:256] / max(pooled[g, 256], tiny).
"""

import os
import sys

import numpy as np

for _p in ("/opt/trn_rl_repo",):
    if _p not in sys.path and os.path.isdir(_p):
        sys.path.insert(0, _p)

import ml_dtypes

import concourse.bass as bass
import concourse.tile as tile
from concourse import bacc, mybir
from concourse import bass_utils

N, D, H, G = 500000, 256, 128, 4096
NCORES = 8
GPC = G // NCORES          # graphs per core = 512
NBLK = 4                   # graph-blocks per core
BLKG = GPC // NBLK         # graphs per block = 128
P = 128                    # partition / chunk size
DP = D + 1                 # node-major row width (ones col appended)
W1SCALE = 64.0             # host multiplies W1 by this before fp8 cast

BF16 = mybir.dt.bfloat16
F32 = mybir.dt.float32
FP8 = mybir.dt.float8e4
NP_BF16 = ml_dtypes.bfloat16
NP_FP8 = ml_dtypes.float8_e4m3fn

LAST_RESULT = None  # test.py reads exec_time_ns / profile from here


# ---------------------------------------------------------------- host plan

def make_plan(batch):
    """Compute the uniform chunk layout from the sorted graph ids."""
    batch = np.asarray(batch)
    seg = np.searchsorted(batch, np.arange(G + 1), side="left")  # [G+1]
    counts = np.zeros((NCORES, NBLK), dtype=np.int64)
    for c in range(NCORES):
        for b in range(NBLK):
            g0 = c * GPC + b * BLKG
            counts[c, b] = seg[g0 + BLKG] - seg[g0]
    cpb = int(np.ceil(counts.max() / P))
    if cpb % 2:
        cpb += 1               # CH = 4*cpb must be divisible by 8
    ch = NBLK * cpb            # chunks per core
    return seg, counts, cpb, ch


def build_inputs(x, batch, W1, b1, W2, seg, cpb, ch):
    """Build the 8 per-core input maps (all shipped data)."""
    x = np.asarray(x, dtype=np.float32)
    batch = np.asarray(batch)
    x_bf = x.astype(NP_BF16)
    x_f8 = x.astype(NP_FP8)
    w1_f8 = (np.asarray(W1, dtype=np.float32) * W1SCALE)  # [D, H]
    # packed k-tiles: [128, 2, H] -> [128, 2*H]
    w1_pk = np.concatenate([w1_f8[0:P, :], w1_f8[P : 2 * P, :]], axis=1)
    w1_pk = w1_pk.astype(NP_FP8)
    b1_f = np.asarray(b1).astype(np.float32).reshape(H, 1)
    w2_f = np.asarray(W2).astype(NP_BF16).reshape(H, 1)
    iota = np.broadcast_to(
        np.arange(P, dtype=np.float32), (P, P)
    ).astype(NP_BF16)  # iota[p, f] = f (0..127 exact in bf16)

    n_g8 = ch // 8
    in_maps = []
    for c in range(NCORES):
        xs = np.zeros((ch * P, DP), dtype=NP_BF16)
        xs[:, D] = 1.0
        xtp = np.zeros((2, P, ch * P), dtype=NP_FP8)
        rel = np.full(ch * P, -1.0, dtype=np.float32)
        for b in range(NBLK):
            g0 = c * GPC + b * BLKG
            s0, s1 = int(seg[g0]), int(seg[g0 + BLKG])
            n = s1 - s0
            r0 = b * cpb * P
            xs[r0 : r0 + n, 0:D] = x_bf[s0:s1]
            xtp[0, :, r0 : r0 + n] = x_f8[s0:s1, 0:P].T
            xtp[1, :, r0 : r0 + n] = x_f8[s0:s1, P:D].T
            rel[r0 : r0 + n] = (batch[s0:s1] - g0).astype(np.float32)
        blr = np.ascontiguousarray(rel.reshape(ch, P).T)  # [128, CH] f32
        # partition-contiguous layouts: one big descriptor per partition/g8
        # xs_r[g8*P + p, j*DP:...] = x row of node (g8*8 + j)*P + p
        xs_r = np.ascontiguousarray(
            xs.reshape(n_g8, 8, P, DP).transpose(0, 2, 1, 3).reshape(
                n_g8 * P, 8 * DP
            )
        )
        # xt_r[g8*P + d, t*NW:...] = xtp[t, d, g8 node window]
        xt_r = np.ascontiguousarray(
            xtp.reshape(2, P, n_g8, 8 * P).transpose(2, 1, 0, 3).reshape(
                n_g8 * P, 2 * 8 * P
            )
        )
        in_maps.append(
            {
                "xs": xs_r,
                "xt": xt_r,
                "blr": blr,
                "w1": w1_pk,
                "b1": b1_f,
                "w2": w2_f,
                "iota": iota,
            }
        )
    return in_maps


# ------------------------------------------------------------- bass program

def build_bass(ch, cpb):
    """Build the SPMD-uniform per-core program."""
    nc = bacc.Bacc(
        "TRN2",
        target_bir_lowering=False,
        debug=False,
        num_devices=NCORES,
    )
    xs = nc.dram_tensor(
        "xs", [ch * P // 8, 8 * DP], BF16, kind="ExternalInput"
    ).ap()
    xt = nc.dram_tensor(
        "xt", [ch * P // 8, 16 * P], FP8, kind="ExternalInput"
    ).ap()
    blr = nc.dram_tensor("blr", [P, ch], F32, kind="ExternalInput").ap()
    w1 = nc.dram_tensor("w1", [P, 2 * H], FP8, kind="ExternalInput").ap()
    b1 = nc.dram_tensor("b1", [H, 1], F32, kind="ExternalInput").ap()
    w2 = nc.dram_tensor("w2", [H, 1], BF16, kind="ExternalInput").ap()
    iota = nc.dram_tensor("iota", [P, P], BF16, kind="ExternalInput").ap()
    out = nc.dram_tensor("out", [GPC, D], F32, kind="ExternalOutput").ap()

    n_g8 = ch // 8  # 8-chunk groups
    NW = 8 * P      # nodes per g8 = 1024

    with tile.TileContext(nc) as tc:
        with (
            tc.tile_pool(name="consts", bufs=1) as cpool,
            tc.tile_pool(name="xb", bufs=8) as xbpool,
            tc.tile_pool(name="xt", bufs=6) as xtpool,
            tc.tile_pool(name="hsb", bufs=3) as hsbpool,
            tc.tile_pool(name="e8", bufs=4) as epool,
            tc.tile_pool(name="oh", bufs=40) as ohpool,
            tc.tile_pool(name="outsb", bufs=2) as outpool,
            tc.tile_pool(name="acc", bufs=1, space="PSUM") as accpool,
            tc.tile_pool(name="hps", bufs=2, space="PSUM") as hpspool,
            tc.tile_pool(name="lg", bufs=2, space="PSUM") as lgpool,
        ):
            # ---- constants into SBUF
            w1_sb = cpool.tile([P, 2 * H], FP8, tag="w1")
            b1_sb = cpool.tile([H, 1], F32, tag="b1")
            w2_sb = cpool.tile([H, 1], BF16, tag="w2")
            io_sb = cpool.tile([P, P], BF16, tag="iota")
            blr_sb = cpool.tile([P, ch], F32, tag="blr")
            nc.sync.dma_start(w1_sb[:], w1[:])
            nc.sync.dma_start(b1_sb[:], b1[:])
            nc.sync.dma_start(w2_sb[:], w2[:])
            nc.sync.dma_start(io_sb[:], iota[:])
            nc.sync.dma_start(blr_sb[:], blr[:])

            # ---- persistent accumulators (PSUM): [g, 0:256]=pooled, 256=denom
            pooled = [
                accpool.tile([P, DP], F32, tag=f"pool{b}", name=f"pool{b}")
                for b in range(NBLK)
            ]



            def flush_pooled(items):
                # pooled[g, 0:257] += oh.T @ [x | 1]  (one group late so the
                # PE has MLP work between logits and the pooled matmuls)
                for oh, xbt, j8, c in items:
                    b = c // cpb
                    first = c == b * cpb
                    last = c == (b + 1) * cpb - 1
                    nc.tensor.matmul(
                        pooled[b][:],
                        oh[:],
                        xbt[:, j8 * DP : (j8 + 1) * DP],
                        start=first,
                        stop=last,
                    )

            # 5-stage software pipeline over g8 groups, one stage per
            # iteration t (group g runs stage s at t = g + s):
            #   s0 DMA; s2 MLP+tanh; s3 logits+exp; s4 oh build; s5 pooled.
            # Cross-engine dependencies are >= 1 full period old when
            # consumed; pooled matmuls (which never stall) head the PE queue
            # so logits behind them get a full segment of slack for their
            # tanh; exp runs in halves so oh builds can start earlier.
            w13 = w1_sb[:].rearrange("p (t m) -> p t m", t=2)
            hsb_q = []
            xb_by_g = {}
            e8_by_g = {}
            xt_q = []
            oh_q = []

            def build_oh(g8, j8s):
                e8_g = e8_by_g[g8]
                xb_g = xb_by_g[g8]
                for j8 in j8s:
                    c = g8 * 8 + j8
                    oh = ohpool.tile([P, P], BF16)
                    # oh[i, g] = (iota[g] == rel_gid[i]) * e[i]
                    nc.vector.tensor_scalar(
                        oh[:],
                        io_sb[:],
                        blr_sb[:, c : c + 1],
                        e8_g[:, j8 : j8 + 1],
                        mybir.AluOpType.is_equal,
                        mybir.AluOpType.mult,
                    )
                    oh_q.append((oh, xb_g, j8, c))

            for t in range(n_g8 + 5):
                if t < n_g8:
                    xb = xbpool.tile([P, 8 * DP], BF16)
                    nc.sync.dma_start(xb[:], xs[t * P : (t + 1) * P, :])
                    xb_by_g[t] = xb
                    xt_sb = xtpool.tile([P, 2 * NW], FP8)
                    nc.sync.dma_start(xt_sb[:], xt[t * P : (t + 1) * P, :])
                    xt_q.append(xt_sb)
                # MLP + tanh for group t-2 (xt two periods old); MLP heads the PE
                # queue so tanh finishes early; exps pack in behind tanh
                # on the Scalar queue exactly when their logits arrive
                if 2 <= t < n_g8 + 2:
                    xt_g = xt_q.pop(0)
                    xt3 = xt_g[:].rearrange("p (t n) -> p t n", t=2)
                    hsb = hsbpool.tile([P, NW], BF16)
                    for half in range(2):
                        hps = hpspool.tile([P, 512], F32)
                        nc.tensor.matmul(
                            hps[:],
                            w13,
                            xt3[:, :, half * 512 : (half + 1) * 512],
                            start=True,
                            stop=True,
                            perf_mode=mybir.MatmulPerfMode.DoubleRow,
                        )
                        nc.scalar.activation(
                            hsb[:, half * 512 : (half + 1) * 512], hps[:],
                            mybir.ActivationFunctionType.Tanh,
                            bias=b1_sb[:], scale=1.0 / W1SCALE,
                        )
                    hsb_q.append(hsb)
                # pooled matmuls (ohs one period old, never stall) head the
                # PE queue
                while oh_q:
                    flush_pooled([oh_q.pop(0)])
                # oh h1-half of group t-4: e8 h1 is one period old, so the
                # DVE starts at period begin instead of behind this period's
                # exp
                if 4 <= t < n_g8 + 4:
                    build_oh(t - 4, range(4, 8))
                    del e8_by_g[t - 4]
                    del xb_by_g[t - 4]
                # logits + exp for group t-3 (hsb one period old); exp in
                # halves, emitted before tanh so it heads the Scalar queue
                if 3 <= t < n_g8 + 3:
                    hsb_g = hsb_q.pop(0)
                    lg = lgpool.tile([P, 8], F32)
                    e8 = epool.tile([P, 8], F32)
                    for j8 in range(8):
                        nc.tensor.matmul(
                            lg[:, j8 : j8 + 1],
                            hsb_g[:, j8 * P : (j8 + 1) * P],
                            w2_sb[:],
                            start=True,
                            stop=True,
                        )
                        if j8 == 3 or j8 == 7:
                            nc.scalar.activation(
                                e8[:, j8 - 3 : j8 + 1],
                                lg[:, j8 - 3 : j8 + 1],
                                mybir.ActivationFunctionType.Exp,
                            )
                    e8_by_g[t - 3] = e8
                    # oh h0-half right after exp h0 (same period)
                    build_oh(t - 3, range(0, 4))
            while oh_q:
                flush_pooled([oh_q.pop(0)])

            # ---- epilogue: out[g] = pooled[g, 0:256] / max(denom[g], tiny)
            for b in range(NBLK):
                dmax = outpool.tile([P, 1], F32, tag="dmax")
                rec = outpool.tile([P, 1], F32, tag="rec")
                nc.vector.tensor_scalar_max(dmax[:], pooled[b][:, D : D + 1], 1e-30)
                nc.vector.reciprocal(rec[:], dmax[:])
                osb = outpool.tile([P, D], F32, tag="osb")
                nc.scalar.mul(osb[:], pooled[b][:, 0:D], rec[:])
                nc.sync.dma_start(out[b * P : (b + 1) * P, :], osb[:])

    nc.compile()
    return nc


# ----------------------------------------------------------------- kernel()

def kernel(**inputs):
    global LAST_RESULT
    x = np.asarray(inputs["x"])
    batch = np.asarray(inputs["batch"])
    W1 = np.asarray(inputs["W1"])
    b1 = np.asarray(inputs["b1"])
    W2 = np.asarray(inputs["W2"])
    # b2 cancels in the softmax; unused.

    seg, counts, cpb, ch = make_plan(batch)
    in_maps = build_inputs(x, batch, W1, b1, W2, seg, cpb, ch)
    nc = build_bass(ch, cpb)
    res = bass_utils.run_bass_kernel_spmd(
        nc, in_maps, list(range(NCORES))
    )
    LAST_RESULT = res
    out = np.concatenate(
        [np.asarray(res.results[c]["out"]) for c in range(NCORES)], axis=0
    )
    return out.astype(np.float32)


# revision 49
# speedup vs baseline: 1.2048x; 1.2048x over previous
"""BaseAttentionPooling Trainium2 kernel.

reference:
    h = tanh(x @ W1 + b1)            # [N, H]
    logits = (h @ W2 + b2)[:, 0]     # [N]
    per-graph softmax over sorted `batch`, pooled = seg_sum(x * w)  # [G, D]

Strategy (data-parallel over graphs, 8 cores, SPMD-identical program):
  - 512 graphs/core, split into 4 blocks of 128 graphs.
  - Host pads each (core, block)'s nodes to `cpb` chunks of 128 nodes
    (cpb = max over all core/blocks, so the program is core-uniform).
  - b2 is dropped: it cancels in the softmax. exp() without max-subtraction:
    |logits| <= ||W2||_1·1 + |b2| is small, exp() safe in fp32.
  - Host ships BOTH x layouts: node-major bf16 (pooling moving operand,
    with a 257th ones column so the denominator rides the pooled matmul)
    and d-major fp8 (MLP moving operand, 2 k-tile planes for DoubleRow).
  - Per g8 group (8 chunks = 1024 nodes):
      hT = W1.T @ xT via 2 fp8 DoubleRow matmuls (K=256 folded, N=512 each)
      tanh(+b1, x1/W1SCALE) on ACT -> hsb bf16
      logits: 8 matmuls (hsb chunk stationary, W2 moving, N=1)
      exp -> e8 [128, 8]
      oh[i,g] = (iota==rel_gid[i])*e[i] built alternately on DVE / GpSimd
      pooled_blk[g, 0:257] += oh.T @ [x | 1] accumulated in PSUM over the
      block's chunks (col 256 = denominator), flushed one g8 late so the
      PE pipeline stays fed.
  - Epilogue: out[g] = pooled[g, 0:256] / max(pooled[g, 256], tiny).
"""

import os
import sys

import numpy as np

for _p in ("/opt/trn_rl_repo",):
    if _p not in sys.path and os.path.isdir(_p):
        sys.path.insert(0, _p)

import ml_dtypes

import concourse.bass as bass
import concourse.tile as tile
from concourse import bacc, mybir
from concourse import bass_utils

N, D, H, G = 500000, 256, 128, 4096
NCORES = 8
GPC = G // NCORES          # graphs per core = 512
NBLK = 4                   # graph-blocks per core
BLKG = GPC // NBLK         # graphs per block = 128
P = 128                    # partition / chunk size
DP = D + 1                 # node-major row width (ones col appended)
W1SCALE = 64.0             # host multiplies W1 by this before fp8 cast

BF16 = mybir.dt.bfloat16
F32 = mybir.dt.float32
FP8 = mybir.dt.float8e4
NP_BF16 = ml_dtypes.bfloat16
NP_FP8 = ml_dtypes.float8_e4m3fn

LAST_RESULT = None  # test.py reads exec_time_ns / profile from here


# ---------------------------------------------------------------- host plan

def make_plan(batch):
    """Compute the uniform chunk layout from the sorted graph ids."""
    batch = np.asarray(batch)
    seg = np.searchsorted(batch, np.arange(G + 1), side="left")  # [G+1]
    counts = np.zeros((NCORES, NBLK), dtype=np.int64)
    for c in range(NCORES):
        for b in range(NBLK):
            g0 = c * GPC + b * BLKG
            counts[c, b] = seg[g0 + BLKG] - seg[g0]
    cpb = int(np.ceil(counts.max() / P))
    if cpb % 2:
        cpb += 1               # CH = 4*cpb must be divisible by 8
    ch = NBLK * cpb            # chunks per core
    return seg, counts, cpb, ch


def build_inputs(x, batch, W1, b1, W2, seg, cpb, ch):
    """Build the 8 per-core input maps (all shipped data)."""
    x = np.asarray(x, dtype=np.float32)
    batch = np.asarray(batch)
    x_bf = x.astype(NP_BF16)
    x_f8 = x.astype(NP_FP8)
    w1_f8 = (np.asarray(W1, dtype=np.float32) * W1SCALE)  # [D, H]
    # packed k-tiles: [128, 2, H] -> [128, 2*H]
    w1_pk = np.concatenate([w1_f8[0:P, :], w1_f8[P : 2 * P, :]], axis=1)
    w1_pk = w1_pk.astype(NP_FP8)
    b1_f = np.asarray(b1).astype(np.float32).reshape(H, 1)
    w2_f = np.asarray(W2).astype(NP_BF16).reshape(H, 1)
    iota = np.broadcast_to(
        np.arange(P, dtype=np.float32), (P, P)
    ).astype(NP_BF16)  # iota[p, f] = f (0..127 exact in bf16)

    n_g8 = ch // 8
    in_maps = []
    for c in range(NCORES):
        xs = np.zeros((ch * P, DP), dtype=NP_BF16)
        xs[:, D] = 1.0
        xtp = np.zeros((2, P, ch * P), dtype=NP_FP8)
        rel = np.full(ch * P, -1.0, dtype=np.float32)
        for b in range(NBLK):
            g0 = c * GPC + b * BLKG
            s0, s1 = int(seg[g0]), int(seg[g0 + BLKG])
            n = s1 - s0
            r0 = b * cpb * P
            xs[r0 : r0 + n, 0:D] = x_bf[s0:s1]
            xtp[0, :, r0 : r0 + n] = x_f8[s0:s1, 0:P].T
            xtp[1, :, r0 : r0 + n] = x_f8[s0:s1, P:D].T
            rel[r0 : r0 + n] = (batch[s0:s1] - g0).astype(np.float32)
        blr = np.ascontiguousarray(rel.reshape(ch, P).T)  # [128, CH] f32
        # partition-contiguous layouts: one big descriptor per partition/g8
        # xs_r[g8*P + p, j*DP:...] = x row of node (g8*8 + j)*P + p
        xs_r = np.ascontiguousarray(
            xs.reshape(n_g8, 8, P, DP).transpose(0, 2, 1, 3).reshape(
                n_g8 * P, 8 * DP
            )
        )
        # xt_r[g8*P + d, t*NW:...] = xtp[t, d, g8 node window]
        xt_r = np.ascontiguousarray(
            xtp.reshape(2, P, n_g8, 8 * P).transpose(2, 1, 0, 3).reshape(
                n_g8 * P, 2 * 8 * P
            )
        )
        in_maps.append(
            {
                "xs": xs_r,
                "xt": xt_r,
                "blr": blr,
                "w1": w1_pk,
                "b1": b1_f,
                "w2": w2_f,
                "iota": iota,
            }
        )
    return in_maps


# ------------------------------------------------------------- bass program

def build_bass(ch, cpb):
    """Build the SPMD-uniform per-core program."""
    nc = bacc.Bacc(
        "TRN2",
        target_bir_lowering=False,
        debug=False,
        num_devices=NCORES,
    )
    xs = nc.dram_tensor(
        "xs", [ch * P // 8, 8 * DP], BF16, kind="ExternalInput"
    ).ap()
    xt = nc.dram_tensor(
        "xt", [ch * P // 8, 16 * P], FP8, kind="ExternalInput"
    ).ap()
    blr = nc.dram_tensor("blr", [P, ch], F32, kind="ExternalInput").ap()
    w1 = nc.dram_tensor("w1", [P, 2 * H], FP8, kind="ExternalInput").ap()
    b1 = nc.dram_tensor("b1", [H, 1], F32, kind="ExternalInput").ap()
    w2 = nc.dram_tensor("w2", [H, 1], BF16, kind="ExternalInput").ap()
    iota = nc.dram_tensor("iota", [P, P], BF16, kind="ExternalInput").ap()
    out = nc.dram_tensor("out", [GPC, D], F32, kind="ExternalOutput").ap()

    n_g8 = ch // 8  # 8-chunk groups
    NW = 8 * P      # nodes per g8 = 1024

    with tile.TileContext(nc) as tc:
        with (
            tc.tile_pool(name="consts", bufs=1) as cpool,
            tc.tile_pool(name="xb", bufs=8) as xbpool,
            tc.tile_pool(name="xt", bufs=6) as xtpool,
            tc.tile_pool(name="hsb", bufs=3) as hsbpool,
            tc.tile_pool(name="e8", bufs=4) as epool,
            tc.tile_pool(name="oh", bufs=40) as ohpool,
            tc.tile_pool(name="outsb", bufs=2) as outpool,
            tc.tile_pool(name="acc", bufs=1, space="PSUM") as accpool,
            tc.tile_pool(name="hps", bufs=2, space="PSUM") as hpspool,
            tc.tile_pool(name="lg", bufs=2, space="PSUM") as lgpool,
        ):
            # ---- constants into SBUF
            w1_sb = cpool.tile([P, 2 * H], FP8, tag="w1")
            b1_sb = cpool.tile([H, 1], F32, tag="b1")
            w2_sb = cpool.tile([H, 1], BF16, tag="w2")
            io_sb = cpool.tile([P, P], BF16, tag="iota")
            blr_sb = cpool.tile([P, ch], F32, tag="blr")
            nc.sync.dma_start(w1_sb[:], w1[:])
            nc.sync.dma_start(b1_sb[:], b1[:])
            nc.sync.dma_start(w2_sb[:], w2[:])
            nc.sync.dma_start(io_sb[:], iota[:])
            nc.sync.dma_start(blr_sb[:], blr[:])

            # ---- persistent accumulators (PSUM): [g, 0:256]=pooled, 256=denom
            pooled = [
                accpool.tile([P, DP], F32, tag=f"pool{b}", name=f"pool{b}")
                for b in range(NBLK)
            ]



            def flush_pooled(items):
                # pooled[g, 0:257] += oh.T @ [x | 1]  (one group late so the
                # PE has MLP work between logits and the pooled matmuls)
                for oh, xbt, j8, c in items:
                    b = c // cpb
                    first = c == b * cpb
                    last = c == (b + 1) * cpb - 1
                    nc.tensor.matmul(
                        pooled[b][:],
                        oh[:],
                        xbt[:, j8 * DP : (j8 + 1) * DP],
                        start=first,
                        stop=last,
                    )

            # 5-stage software pipeline over g8 groups, one stage per
            # iteration t (group g runs stage s at t = g + s):
            #   s0 DMA; s2 MLP+tanh; s3 logits+exp; s4 oh build; s5 pooled.
            # Cross-engine dependencies are >= 1 full period old when
            # consumed; pooled matmuls (which never stall) head the PE queue
            # so logits behind them get a full segment of slack for their
            # tanh; exp runs in halves so oh builds can start earlier.
            w13 = w1_sb[:].rearrange("p (t m) -> p t m", t=2)
            hsb_q = []
            xb_by_g = {}
            e8_by_g = {}
            xt_q = []
            oh_q = []

            def build_oh(g8, j8s):
                e8_g = e8_by_g[g8]
                xb_g = xb_by_g[g8]
                for j8 in j8s:
                    c = g8 * 8 + j8
                    oh = ohpool.tile([P, P], BF16)
                    # oh[i, g] = (iota[g] == rel_gid[i]) * e[i]
                    nc.vector.tensor_scalar(
                        oh[:],
                        io_sb[:],
                        blr_sb[:, c : c + 1],
                        e8_g[:, j8 : j8 + 1],
                        mybir.AluOpType.is_equal,
                        mybir.AluOpType.mult,
                    )
                    oh_q.append((oh, xb_g, j8, c))

            for t in range(n_g8 + 5):
                if t < n_g8:
                    xb = xbpool.tile([P, 8 * DP], BF16)
                    nc.sync.dma_start(xb[:], xs[t * P : (t + 1) * P, :])
                    xb_by_g[t] = xb
                    xt_sb = xtpool.tile([P, 2 * NW], FP8)
                    nc.sync.dma_start(xt_sb[:], xt[t * P : (t + 1) * P, :])
                    xt_q.append(xt_sb)
                # pooled matmuls (ohs one period old, never stall) head the
                # PE queue
                while oh_q:
                    flush_pooled([oh_q.pop(0)])
                # oh h1-half of group t-4: e8 h1 is one period old, so the
                # DVE starts at period begin instead of behind this period's
                # exp
                if 4 <= t < n_g8 + 4:
                    build_oh(t - 4, range(4, 8))
                    del e8_by_g[t - 4]
                    del xb_by_g[t - 4]
                # logits + exp for group t-3 (hsb one period old); exp in
                # halves, emitted before tanh so it heads the Scalar queue
                if 3 <= t < n_g8 + 3:
                    hsb_g = hsb_q.pop(0)
                    lg = lgpool.tile([P, 8], F32)
                    e8 = epool.tile([P, 8], F32)
                    for j8 in range(8):
                        nc.tensor.matmul(
                            lg[:, j8 : j8 + 1],
                            hsb_g[:, j8 * P : (j8 + 1) * P],
                            w2_sb[:],
                            start=True,
                            stop=True,
                        )
                        if j8 == 3 or j8 == 7:
                            nc.scalar.activation(
                                e8[:, j8 - 3 : j8 + 1],
                                lg[:, j8 - 3 : j8 + 1],
                                mybir.ActivationFunctionType.Exp,
                            )
                    e8_by_g[t - 3] = e8
                    # oh h0-half right after exp h0 (same period)
                    build_oh(t - 3, range(0, 4))
                # MLP + tanh for group t-2 (xt two periods old); tanh sits
                # behind exp in the Scalar queue
                if 2 <= t < n_g8 + 2:
                    xt_g = xt_q.pop(0)
                    xt3 = xt_g[:].rearrange("p (t n) -> p t n", t=2)
                    hsb = hsbpool.tile([P, NW], BF16)
                    for half in range(2):
                        hps = hpspool.tile([P, 512], F32)
                        nc.tensor.matmul(
                            hps[:],
                            w13,
                            xt3[:, :, half * 512 : (half + 1) * 512],
                            start=True,
                            stop=True,
                            perf_mode=mybir.MatmulPerfMode.DoubleRow,
                        )
                        nc.scalar.activation(
                            hsb[:, half * 512 : (half + 1) * 512], hps[:],
                            mybir.ActivationFunctionType.Tanh,
                            bias=b1_sb[:], scale=1.0 / W1SCALE,
                        )
                    hsb_q.append(hsb)
            while oh_q:
                flush_pooled([oh_q.pop(0)])

            # ---- epilogue: out[g] = pooled[g, 0:256] / max(denom[g], tiny)
            for b in range(NBLK):
                dmax = outpool.tile([P, 1], F32, tag="dmax")
                rec = outpool.tile([P, 1], F32, tag="rec")
                nc.vector.tensor_scalar_max(dmax[:], pooled[b][:, D : D + 1], 1e-30)
                nc.vector.reciprocal(rec[:], dmax[:])
                osb = outpool.tile([P, D], F32, tag="osb")
                nc.scalar.mul(osb[:], pooled[b][:, 0:D], rec[:])
                nc.sync.dma_start(out[b * P : (b + 1) * P, :], osb[:])

    nc.compile()
    return nc


# ----------------------------------------------------------------- kernel()

def kernel(**inputs):
    global LAST_RESULT
    x = np.asarray(inputs["x"])
    batch = np.asarray(inputs["batch"])
    W1 = np.asarray(inputs["W1"])
    b1 = np.asarray(inputs["b1"])
    W2 = np.asarray(inputs["W2"])
    # b2 cancels in the softmax; unused.

    seg, counts, cpb, ch = make_plan(batch)
    in_maps = build_inputs(x, batch, W1, b1, W2, seg, cpb, ch)
    nc = build_bass(ch, cpb)
    res = bass_utils.run_bass_kernel_spmd(
        nc, in_maps, list(range(NCORES))
    )
    LAST_RESULT = res
    out = np.concatenate(
        [np.asarray(res.results[c]["out"]) for c in range(NCORES)], axis=0
    )
    return out.astype(np.float32)
